# revision 27
# baseline (speedup 1.0000x reference)
"""Trainium2 Bass kernel for nn_PolicyNet_78365973283198 (GNN message passing).

Computation (reference):
    tempHS = tanh(state_HS @ W_fs + b_fs)          # [N, 128]
    u0     = tempHS @ W_fp + b_fp                  # [N]
    uk[e]  = <tempHS[seg[e]], hats[e]>             # [E]  (seg sorted)
    out    = sigmoid(concat([u0, uk]))             # [N + E]

Design: data-parallel over nodes on 8 cores (6250 nodes each). Nodes are
sorted by degree (descending); the degree sequence is canonicalized to the
per-rank max across cores so ONE program serves all 8 cores (~4% pad).
Blocks of 128 chunks share a uniform degree d_b (block max).

uk via ALL-PAIRS matmuls: per block, phase A produces thT [d=128, n=128]
(fp16, W hi/lo split for precision). For each group of C chunks
(C = 32/16/8 so W = C*d <= 512), ONE matmul out[c, s] = <th[g*C+c], hats[s]>
against the RAW hats tile (packed [E_DIM, slots] fp16 on host) computes every
needed dot product at 1 PE cycle/slot -- no expansion matmul, no DVE multiply,
no reduce matmul (the old pipeline cost ~2x PE + a DVE pass). The needed
values form a per-group diagonal band; Act/DVE copy the PSUM group tiles into
an SBUF stage and a single 3-dim "diagonal" DMA per block
([(L+d, C), (W, ng), (1, d)] -- partition-crossing stride on dim 0 only,
offset < row length, C <= 42: all hardware-validated) extracts the band
straight to DRAM. u0 rides on the same loaded weights as an N=1 matmul.
Host applies sigmoid to uk during unpack; u0 is sigmoided on device.
"""

import numpy as np

PHASEA_HILO = False     # hi/lo W split in phase A (2x matmuls, +precision)
N_NODES = 50000
N_EDGES = 600000
IN_DIM = 512
E_DIM = 128
NC = 8
NPER = N_NODES // NC
SEGCOLS = 4096          # hats segment width (cols) = 1MB fp16 per DMA
HB_BUFS = 6             # hats segment ring depth
HB_PREF = 4             # segments to prefetch ahead of consumption
STGW = 4352             # uk stage width (f32 cols); caps block degree at 34


def _f16(x):
    return np.ascontiguousarray(x, dtype=np.float16)


def _group_c(d):
    if d <= 16:
        return 32
    if d <= 32:
        return 16
    if d <= 64:
        return 8
    raise AssertionError(f"degree {d} > 64 unsupported")


def _zigzag(n):
    lo, hi = 0, n - 1
    out = []
    while lo <= hi:
        out.append(lo)
        if hi != lo:
            out.append(hi)
        lo += 1
        hi -= 1
    return out


class Geom:
    """Canonical (core-independent) program geometry."""

    def __init__(self, D):
        # D: canonical per-chunk degrees, len NPER, sorted descending
        NBLK = -(-NPER // 128)
        self.NR = -(-NBLK // 4)
        self.NBLK4 = 4 * self.NR
        self.NCHUNK = 128 * self.NBLK4
        Dp = np.zeros(self.NCHUNK, np.int64)
        Dp[:NPER] = D
        self.d_b = [int(Dp[128 * b]) for b in range(self.NBLK4)]
        assert 128 * max(self.d_b) <= STGW, f"max block degree {max(self.d_b)}"

        # zig-zag BLOCK order: alternate compute-dense (big d) and
        # overhead-dense (small d) blocks so DMA demand and per-block
        # engine overhead stay uniform across the run
        self.block_perm = _zigzag(self.NBLK4)
        proc = self.block_perm
        self.blocks = [None] * self.NBLK4
        seg_i, cur, ukoff = 0, 0, 0
        for b in proc:
            d = self.d_b[b]
            if d == 0:
                continue
            C = _group_c(d)
            ng = 128 // C
            W = C * d
            gplace = []
            for g in range(ng):
                if cur + W > SEGCOLS:
                    seg_i += 1
                    cur = 0
                gplace.append((seg_i, cur))
                cur += W
            self.blocks[b] = (d, C, ng, W, gplace, ukoff)
            ukoff += 128 * d
        self.NSEG = seg_i + 1
        self.SUK = max(ukoff, 1)


def emit(nc, t, geom):
    import concourse.tile as tile
    from concourse import mybir
    from concourse.ap import AP

    fp16 = mybir.dt.float16
    f32 = mybir.dt.float32
    Act = mybir.ActivationFunctionType

    wcat_d, bfs_d, wfp_d, bfp_d = t["wcat"], t["bfs"], t["wfp"], t["bfp"]
    state_d, hats_d = t["state_p"], t["hats_p"]
    uk_d, u0_d = t["uk_o"], t["u0_o"]
    NR, NSEG, NBLK4 = geom.NR, geom.NSEG, geom.NBLK4

    with tile.TileContext(nc) as tc:
        with (
            tc.tile_pool(name="const", bufs=1) as cpool,
            tc.tile_pool(name="perst", bufs=1) as ppool,
            tc.tile_pool(name="st", bufs=4) as stpool,
            tc.tile_pool(name="th", bufs=3) as thpool,
            tc.tile_pool(name="hat", bufs=HB_BUFS) as hpool,
            tc.tile_pool(name="stg", bufs=6) as sgpool,
            tc.tile_pool(name="psA", bufs=2, space="PSUM") as psA,
            tc.tile_pool(name="psK", bufs=5, space="PSUM") as psK,
            tc.tile_pool(name="psU", bufs=1, space="PSUM") as psU,
        ):
            wcat = cpool.tile([128, (8 if PHASEA_HILO else 4) * 128], fp16, tag="wcat")
            nc.sync.dma_start(wcat[:], wcat_d[:])
            bfs = cpool.tile([128, 1], fp16, tag="bfs")
            nc.sync.dma_start(bfs[:], bfs_d[:])
            wfp = cpool.tile([128, 1], fp16, tag="wfp")
            nc.sync.dma_start(wfp[:], wfp_d[:])
            bfp = cpool.tile([128, 1], f32, tag="bfp")
            nc.sync.dma_start(bfp[:], bfp_d[:])

            u0acc = psU.tile([128, NBLK4], f32, tag="u0acc")

            # hats segment ring with explicit prefetch emission
            hseg = {}
            next_seg = [0]

            def fetch_seg():
                s = next_seg[0]
                if s >= NSEG:
                    return
                tile_ = hpool.tile([128, SEGCOLS], fp16, tag="hseg")
                # SWDGE: keeps bulk loads off the sync/scalar queues so the
                # diag DMAs and Act compute never stall behind a buffer wait
                nc.gpsimd.dma_start(tile_[:], hats_d[s])
                hseg[s] = tile_
                next_seg[0] += 1

            st_tiles = {}
            next_st = [0]

            def fetch_st():
                k = next_st[0]
                if k >= NR:
                    return
                tile_ = stpool.tile([128, 2048], fp16, tag="st")
                # same SWDGE queue as hats: FIFO order == consumption order,
                # so the urgent state round is never starved by hats bulk
                nc.gpsimd.dma_start(tile_[:], state_d[k])
                st_tiles[k] = tile_
                next_st[0] += 1

            # state first: the first phase-A round must not queue behind the
            # hats prefetch burst (SDMA fair-shares rings at packet level)
            fetch_st()
            fetch_st()
            for _ in range(min(HB_PREF + 1, NSEG)):
                fetch_seg()

            cp_i = [0]          # copy engine round robin

            def copy(out_ap, in_ap):
                # weight DVE slightly more than Act (Act also does tanh+DMA)
                k = cp_i[0] % 5
                cp_i[0] += 1
                if k in (0, 1, 3):
                    nc.vector.tensor_scalar_add(out=out_ap, in0=in_ap, scalar1=0.0)
                else:
                    nc.scalar.activation(out_ap, in_ap, Act.Copy)

            for k in range(NR):
                rblocks = geom.block_perm[4 * k:4 * k + 4]
                npass = 8 if PHASEA_HILO else 4
                thT4 = thpool.tile([128, 512], fp16, tag="thT4")
                stb = st_tiles.pop(k)
                fetch_st()
                tp = psA.tile([128, 512], f32, tag="tp")
                for i in range(npass):      # (hi/lo, c4) combos
                    c4 = i % 4
                    nc.tensor.matmul(
                        tp[:],
                        lhsT=wcat[:, i * 128:(i + 1) * 128],
                        rhs=stb[:, c4 * 512:(c4 + 1) * 512],
                        start=(i == 0), stop=(i == npass - 1),
                    )
                nc.scalar.activation(thT4[:], tp[:], Act.Tanh,
                                     bias=bfs[:, 0:1])

                for b4, b in enumerate(rblocks):
                    thT = thT4[:, b4 * 128:(b4 + 1) * 128]
                    nc.tensor.matmul(u0acc[:, b:b + 1], lhsT=thT, rhs=wfp[:],
                                     start=True, stop=True)
                    blk = geom.blocks[b]
                    if blk is None:
                        continue
                    d, C, ng, W, gplace, ukoff = blk
                    LB = ng * W     # = 128*d
                    stage = sgpool.tile([128, STGW], f32, tag="stage")
                    for g in range(ng):
                        s, off = gplace[g]
                        while next_seg[0] <= s + HB_PREF:
                            if next_seg[0] >= NSEG:
                                break
                            fetch_seg()
                        hs = hseg[s]
                        pk = psK.tile([128, 512], f32, tag="pk")
                        nc.tensor.matmul(
                            pk[0:C, 0:W],
                            lhsT=thT[:, g * C:(g + 1) * C],
                            rhs=hs[:, off:off + W],
                            start=True, stop=True)
                        copy(stage[0:C, g * W:(g + 1) * W], pk[0:C, 0:W])
                    sap = stage[:]
                    diag = AP(sap.tensor, sap.offset,
                              [(STGW + d, C), (W, ng), (1, d)])
                    nc.sync.dma_start(uk_d[0:1, ukoff:ukoff + 128 * d], diag)

            u0sb = ppool.tile([128, NBLK4], f32, tag="u0sb")
            nc.scalar.activation(u0sb[:], u0acc[:], Act.Sigmoid, bias=bfp[:, 0:1])
            nc.sync.dma_start(u0_d[:], u0sb[:])
    return []


def build_nc(geom):
    import concourse.bass as bass
    from concourse import mybir

    fp16 = mybir.dt.float16
    f32 = mybir.dt.float32

    nc = bass.Bass("TRN2", target_bir_lowering=False, debug=False)
    t = {
        "wcat": nc.dram_tensor("wcat", [128, (8 if PHASEA_HILO else 4) * 128], fp16,
                               kind="ExternalInput")[:],
        "bfs": nc.dram_tensor("bfs", [128, 1], fp16, kind="ExternalInput")[:],
        "wfp": nc.dram_tensor("wfp", [128, 1], fp16, kind="ExternalInput")[:],
        "bfp": nc.dram_tensor("bfp", [128, 1], f32, kind="ExternalInput")[:],
        "state_p": nc.dram_tensor("state_p", [geom.NR, 128, 2048], fp16,
                                  kind="ExternalInput"),
        "hats_p": nc.dram_tensor("hats_p", [geom.NSEG, 128, SEGCOLS], fp16,
                                 kind="ExternalInput"),
        "uk_o": nc.dram_tensor("uk_o", [1, geom.SUK], f32,
                               kind="ExternalOutput")[:],
        "u0_o": nc.dram_tensor("u0_o", [128, geom.NBLK4], f32,
                               kind="ExternalOutput")[:],
    }
    emit(nc, t, geom)
    split_multi_waits(nc)
    return nc


def split_multi_waits(nc):
    """This env's walrus encodes at most one sem wait per instruction; hoist
    extras onto standalone EventSemaphore insts immediately before."""
    import concourse.mybir as mybir
    n = 0
    for fn in nc.m.functions:
        for bb in fn.blocks:
            insts = list(bb.instructions)
            if not any(i.sync_info and len(i.sync_info.on_wait) > 1 for i in insts):
                continue
            out = []
            for inst in insts:
                si = inst.sync_info
                if si is not None and len(si.on_wait) > 1:
                    waits = list(si.on_wait)
                    for w in waits[:-1]:
                        n += 1
                        out.append(mybir.InstEventSemaphore(
                            name=f"splitw_{n}_{inst.name}",
                            engine=inst.engine, ins=[], outs=[],
                            sync_info=mybir.SyncInfo(on_wait=[w], on_update=[]),
                        ))
                    inst.sync_info = mybir.SyncInfo(
                        on_wait=[waits[-1]], on_update=list(si.on_update))
                out.append(inst)
            bb.instructions = out
    return n


def prep_inputs(state_HS, hats, seg, W_fs, b_fs, W_fp, b_fp):
    """Shard + pack. Returns (in_maps, geom, maps)."""
    state_HS = np.asarray(state_HS, dtype=np.float32)
    hats = np.asarray(hats, dtype=np.float32)
    seg = np.asarray(seg, dtype=np.int32)
    W_fs = np.asarray(W_fs, dtype=np.float32)
    b_fs = np.asarray(b_fs, dtype=np.float32)
    W_fp = np.asarray(W_fp, dtype=np.float32)
    b_fp = np.asarray(b_fp, dtype=np.float32)

    deg = np.bincount(seg, minlength=N_NODES).astype(np.int64)
    estart = np.concatenate([[0], np.cumsum(deg)[:-1]])

    orders = []
    deg_sorted = np.empty((NC, NPER), np.int64)
    for c in range(NC):
        nodes = np.arange(c * NPER, (c + 1) * NPER)
        o = np.lexsort((nodes, -deg[nodes]))
        orders.append(nodes[o])
        deg_sorted[c] = deg[nodes[o]]
    D = deg_sorted.max(axis=0)
    geom = Geom(D)

    # constants
    w_hi = W_fs.astype(np.float16)
    w_lo = (W_fs.astype(np.float64) - w_hi.astype(np.float64)).astype(np.float16)
    ws = (w_hi, w_lo) if PHASEA_HILO else (w_hi,)
    wcat = np.empty((128, 4 * len(ws), 128), dtype=np.float16)
    for hl, w in enumerate(ws):
        for c4 in range(4):
            wcat[:, hl * 4 + c4, :] = w[c4 * 128:(c4 + 1) * 128, :]
    wcat = wcat.reshape(128, -1)
    bfs_c = _f16(b_fs.reshape(128, 1))
    wfp_c = _f16(W_fp.reshape(128, 1))
    bfp_c = np.full((128, 1), float(b_fp[0]), dtype=np.float32)

    state16 = state_HS.astype(np.float16)
    hats16 = hats.astype(np.float16)

    in_maps, maps = [], []
    for c in range(NC):
        order = orders[c]
        ordp = np.concatenate(
            [order, np.full(geom.NCHUNK - NPER, order[-1], np.int64)])
        degp = np.concatenate(
            [deg_sorted[c], np.zeros(geom.NCHUNK - NPER, np.int64)])

        # state rounds [NR, 128, 2048]
        st_p = np.empty((geom.NR, 128, 2048), np.float16)
        for k in range(geom.NR):
            rblocks = geom.block_perm[4 * k:4 * k + 4]
            nodes512 = np.concatenate(
                [ordp[128 * b:128 * (b + 1)] for b in rblocks])
            arr = state16[nodes512]                      # [512, 512]
            st_p[k] = (arr.reshape(512, 4, 128)
                       .transpose(2, 1, 0).reshape(128, 2048))

        # hats segments [NSEG, 128, SEGCOLS]
        hp = np.zeros((geom.NSEG, 128, SEGCOLS), np.float16)
        for b in range(geom.NBLK4):
            blk = geom.blocks[b]
            if blk is None:
                continue
            d, C, ng, W, gplace, ukoff = blk
            ci = 128 * b + np.arange(128)                # chunk index [ng*C]
            nodesb = ordp[ci].reshape(ng, C)
            degb = degp[ci].reshape(ng, C)
            j = np.arange(d)
            e = estart[nodesb][:, :, None] + j[None, None, :]
            valid = j[None, None, :] < degb[:, :, None]
            eidx = np.where(valid, e, 0)
            vals = hats16[eidx]                          # [ng, C, d, 128]
            for g in range(ng):
                s, off = gplace[g]
                hp[s][:, off:off + W] = vals[g].reshape(W, 128).T
        in_maps.append({
            "wcat": wcat, "bfs": bfs_c, "wfp": wfp_c, "bfp": bfp_c,
            "state_p": st_p,
            "hats_p": np.ascontiguousarray(hp),
        })
        maps.append((ordp, degp, estart[ordp]))
    return in_maps, geom, maps


def assemble(results, geom, maps):
    out = np.empty(N_NODES + N_EDGES, dtype=np.float32)
    for c in range(NC):
        ordp, degp, e0p = maps[c]
        uk = np.asarray(results[c]["uk_o"]).reshape(-1)
        u0 = np.asarray(results[c]["u0_o"])              # [128, NBLK4]
        i = np.arange(NPER)
        out[ordp[:NPER]] = u0[i % 128, i // 128]
        # uk: per block, diag output order is [chunk-in-group, group, slot]
        srcs, dsts = [], []
        for b in range(geom.NBLK4):
            blk = geom.blocks[b]
            if blk is None:
                continue
            d, C, ng, W, gplace, ukoff = blk
            ci = 128 * b + np.arange(128)
            degb = degp[ci]
            cc = np.arange(128)                          # chunk-in-block
            g = cc // C
            r = cc % C
            j = np.arange(d)
            pos = ukoff + r[:, None] * (ng * d) + g[:, None] * d + j[None, :]
            valid = j[None, :] < degb[:, None]
            if not valid.any():
                continue
            # edge index needs global estart; recompute cheaply
            srcs.append(pos[valid])
            e0 = e0p[ci]
            dsts.append((e0[:, None] + j[None, :])[valid])
        if srcs:
            sp = np.concatenate(srcs)
            dp = np.concatenate(dsts)
            out[N_NODES + dp] = 1.0 / (1.0 + np.exp(-uk[sp]))
    return out


def kernel(state_HS, hats, seg, W_fs, b_fs, W_fp, b_fp):
    from concourse.bass_utils import run_bass_kernel_spmd
    in_maps, geom, maps = prep_inputs(
        state_HS, hats, seg, W_fs, b_fs, W_fp, b_fp)
    nc = build_nc(geom)
    res = run_bass_kernel_spmd(nc, in_maps, core_ids=list(range(NC)))
    return assemble(res.results, geom, maps)


# revision 28
# speedup vs baseline: 1.0609x; 1.0609x over previous
"""Trainium2 Bass kernel for nn_PolicyNet_78365973283198 (GNN message passing).

Computation (reference):
    tempHS = tanh(state_HS @ W_fs + b_fs)          # [N, 128]
    u0     = tempHS @ W_fp + b_fp                  # [N]
    uk[e]  = <tempHS[seg[e]], hats[e]>             # [E]  (seg sorted)
    out    = sigmoid(concat([u0, uk]))             # [N + E]

Design: data-parallel over nodes on 8 cores (6250 nodes each). Nodes are
sorted by degree (descending); the degree sequence is canonicalized to the
per-rank max across cores so ONE program serves all 8 cores (~4% pad).
Blocks of 128 chunks share a uniform degree d_b (block max).

uk via ALL-PAIRS matmuls: per block, phase A produces thT [d=128, n=128]
(fp16, W hi/lo split for precision). For each group of C chunks
(C = 32/16/8 so W = C*d <= 512), ONE matmul out[c, s] = <th[g*C+c], hats[s]>
against the RAW hats tile (packed [E_DIM, slots] fp16 on host) computes every
needed dot product at 1 PE cycle/slot -- no expansion matmul, no DVE multiply,
no reduce matmul (the old pipeline cost ~2x PE + a DVE pass). The needed
values form a per-group diagonal band; Act/DVE copy the PSUM group tiles into
an SBUF stage and a single 3-dim "diagonal" DMA per block
([(L+d, C), (W, ng), (1, d)] -- partition-crossing stride on dim 0 only,
offset < row length, C <= 42: all hardware-validated) extracts the band
straight to DRAM. u0 rides on the same loaded weights as an N=1 matmul.
Host applies sigmoid to uk during unpack; u0 is sigmoided on device.
"""

import numpy as np

PHASEA_HILO = False     # hi/lo W split in phase A (2x matmuls, +precision)
N_NODES = 50000
N_EDGES = 600000
IN_DIM = 512
E_DIM = 128
NC = 8
NPER = N_NODES // NC
SEGCOLS = 4096          # hats segment width (cols) = 1MB fp16 per DMA
HB_BUFS = 6             # hats segment ring depth
HB_PREF = 4             # segments to prefetch ahead of consumption
STGW = 4352             # uk stage width (f32 cols); caps block degree at 34


def _f16(x):
    return np.ascontiguousarray(x, dtype=np.float16)


def _group_c(d):
    if d <= 16:
        return 32
    if d <= 32:
        return 16
    if d <= 64:
        return 8
    raise AssertionError(f"degree {d} > 64 unsupported")


def _zigzag(n):
    lo, hi = 0, n - 1
    out = []
    while lo <= hi:
        out.append(lo)
        if hi != lo:
            out.append(hi)
        lo += 1
        hi -= 1
    return out


class Geom:
    """Canonical (core-independent) program geometry."""

    def __init__(self, D):
        # D: canonical per-chunk degrees, len NPER, sorted descending
        NBLK = -(-NPER // 128)
        self.NR = -(-NBLK // 4)
        self.NBLK4 = 4 * self.NR
        self.NCHUNK = 128 * self.NBLK4
        Dp = np.zeros(self.NCHUNK, np.int64)
        Dp[:NPER] = D
        self.d_b = [int(Dp[128 * b]) for b in range(self.NBLK4)]
        assert 128 * max(self.d_b) <= STGW, f"max block degree {max(self.d_b)}"

        # zig-zag BLOCK order: alternate compute-dense (big d) and
        # overhead-dense (small d) blocks so DMA demand and per-block
        # engine overhead stay uniform across the run
        self.block_perm = _zigzag(self.NBLK4)
        proc = self.block_perm
        self.blocks = [None] * self.NBLK4
        seg_i, cur, ukoff = 0, 0, 0
        for b in proc:
            d = self.d_b[b]
            if d == 0:
                continue
            C = _group_c(d)
            ng = 128 // C
            W = C * d
            gplace = []
            for g in range(ng):
                if cur + W > SEGCOLS:
                    seg_i += 1
                    cur = 0
                gplace.append((seg_i, cur))
                cur += W
            self.blocks[b] = (d, C, ng, W, gplace, ukoff)
            ukoff += 128 * d
        self.NSEG = seg_i + 1
        self.SUK = max(ukoff, 1)


def emit(nc, t, geom):
    import concourse.tile as tile
    from concourse import mybir
    from concourse.ap import AP

    fp16 = mybir.dt.float16
    f32 = mybir.dt.float32
    Act = mybir.ActivationFunctionType

    wcat_d, bfs_d, wfp_d, bfp_d = t["wcat"], t["bfs"], t["wfp"], t["bfp"]
    state_d, hats_d = t["state_p"], t["hats_p"]
    uk_d, u0_d = t["uk_o"], t["u0_o"]
    NR, NSEG, NBLK4 = geom.NR, geom.NSEG, geom.NBLK4

    with tile.TileContext(nc) as tc:
        with (
            tc.tile_pool(name="const", bufs=1) as cpool,
            tc.tile_pool(name="perst", bufs=1) as ppool,
            tc.tile_pool(name="st", bufs=4) as stpool,
            tc.tile_pool(name="th", bufs=3) as thpool,
            tc.tile_pool(name="hat", bufs=HB_BUFS) as hpool,
            tc.tile_pool(name="stg", bufs=6) as sgpool,
            tc.tile_pool(name="psA", bufs=2, space="PSUM") as psA,
            tc.tile_pool(name="psK", bufs=5, space="PSUM") as psK,
            tc.tile_pool(name="psU", bufs=1, space="PSUM") as psU,
        ):
            wcat = cpool.tile([128, (8 if PHASEA_HILO else 4) * 128], fp16, tag="wcat")
            nc.sync.dma_start(wcat[:], wcat_d[:])
            bfs = cpool.tile([128, 1], fp16, tag="bfs")
            nc.sync.dma_start(bfs[:], bfs_d[:])
            wfp = cpool.tile([128, 1], fp16, tag="wfp")
            nc.sync.dma_start(wfp[:], wfp_d[:])
            bfp = cpool.tile([128, 1], f32, tag="bfp")
            nc.sync.dma_start(bfp[:], bfp_d[:])

            u0acc = psU.tile([128, NBLK4], f32, tag="u0acc")

            # hats segment ring with explicit prefetch emission
            hseg = {}
            next_seg = [0]

            def fetch_seg():
                s = next_seg[0]
                if s >= NSEG:
                    return
                tile_ = hpool.tile([128, SEGCOLS], fp16, tag="hseg")
                # SWDGE: keeps bulk loads off the sync/scalar queues so the
                # diag DMAs and Act compute never stall behind a buffer wait
                nc.gpsimd.dma_start(tile_[:], hats_d[s])
                hseg[s] = tile_
                next_seg[0] += 1

            st_tiles = {}
            next_st = [0]

            def fetch_st():
                k = next_st[0]
                if k >= NR:
                    return
                tile_ = stpool.tile([128, 2048], fp16, tag="st")
                # same SWDGE queue as hats: FIFO order == consumption order,
                # so the urgent state round is never starved by hats bulk
                nc.gpsimd.dma_start(tile_[:], state_d[k])
                st_tiles[k] = tile_
                next_st[0] += 1

            # state first: the first phase-A round must not queue behind the
            # hats prefetch burst (SDMA fair-shares rings at packet level)
            fetch_st()
            fetch_st()
            for _ in range(min(HB_PREF + 1, NSEG)):
                fetch_seg()

            cp_i = [0]          # copy engine round robin

            def copy(out_ap, in_ap):
                # weight DVE slightly more than Act (Act also does tanh+DMA)
                k = cp_i[0] % 5
                cp_i[0] += 1
                if k in (0, 1, 3):
                    nc.vector.tensor_scalar_add(out=out_ap, in0=in_ap, scalar1=0.0)
                else:
                    nc.scalar.activation(out_ap, in_ap, Act.Copy)

            npass = 8 if PHASEA_HILO else 4
            th4s = {}

            def emit_phase_a(k):
                stb = st_tiles.pop(k)
                fetch_st()
                tp = psA.tile([128, 512], f32, tag="tp")
                for i in range(npass):      # (hi/lo, c4) combos
                    c4 = i % 4
                    nc.tensor.matmul(
                        tp[:],
                        lhsT=wcat[:, i * 128:(i + 1) * 128],
                        rhs=stb[:, c4 * 512:(c4 + 1) * 512],
                        start=(i == 0), stop=(i == npass - 1),
                    )
                thT4 = thpool.tile([128, 512], fp16, tag="thT4")
                nc.scalar.activation(thT4[:], tp[:], Act.Tanh,
                                     bias=bfs[:, 0:1])
                th4s[k] = thT4

            # software pipeline: phase A of round k+1 is emitted BEFORE round
            # k's group matmuls, so when groups stall on a hats segment the
            # Tensor queue still has (already-issued) phase-A work done and
            # round k+1 starts the instant its hats arrive
            emit_phase_a(0)
            for k in range(NR):
                rblocks = geom.block_perm[4 * k:4 * k + 4]
                if k + 1 < NR:
                    emit_phase_a(k + 1)
                thT4 = th4s.pop(k)

                for b4, b in enumerate(rblocks):
                    thT = thT4[:, b4 * 128:(b4 + 1) * 128]
                    nc.tensor.matmul(u0acc[:, b:b + 1], lhsT=thT, rhs=wfp[:],
                                     start=True, stop=True)
                    blk = geom.blocks[b]
                    if blk is None:
                        continue
                    d, C, ng, W, gplace, ukoff = blk
                    LB = ng * W     # = 128*d
                    stage = sgpool.tile([128, STGW], f32, tag="stage")
                    for g in range(ng):
                        s, off = gplace[g]
                        while next_seg[0] <= s + HB_PREF:
                            if next_seg[0] >= NSEG:
                                break
                            fetch_seg()
                        hs = hseg[s]
                        pk = psK.tile([128, 512], f32, tag="pk")
                        nc.tensor.matmul(
                            pk[0:C, 0:W],
                            lhsT=thT[:, g * C:(g + 1) * C],
                            rhs=hs[:, off:off + W],
                            start=True, stop=True)
                        copy(stage[0:C, g * W:(g + 1) * W], pk[0:C, 0:W])
                    sap = stage[:]
                    diag = AP(sap.tensor, sap.offset,
                              [(STGW + d, C), (W, ng), (1, d)])
                    nc.sync.dma_start(uk_d[0:1, ukoff:ukoff + 128 * d], diag)

            u0sb = ppool.tile([128, NBLK4], f32, tag="u0sb")
            nc.scalar.activation(u0sb[:], u0acc[:], Act.Sigmoid, bias=bfp[:, 0:1])
            nc.sync.dma_start(u0_d[:], u0sb[:])
    return []


def build_nc(geom):
    import concourse.bass as bass
    from concourse import mybir

    fp16 = mybir.dt.float16
    f32 = mybir.dt.float32

    nc = bass.Bass("TRN2", target_bir_lowering=False, debug=False)
    t = {
        "wcat": nc.dram_tensor("wcat", [128, (8 if PHASEA_HILO else 4) * 128], fp16,
                               kind="ExternalInput")[:],
        "bfs": nc.dram_tensor("bfs", [128, 1], fp16, kind="ExternalInput")[:],
        "wfp": nc.dram_tensor("wfp", [128, 1], fp16, kind="ExternalInput")[:],
        "bfp": nc.dram_tensor("bfp", [128, 1], f32, kind="ExternalInput")[:],
        "state_p": nc.dram_tensor("state_p", [geom.NR, 128, 2048], fp16,
                                  kind="ExternalInput"),
        "hats_p": nc.dram_tensor("hats_p", [geom.NSEG, 128, SEGCOLS], fp16,
                                 kind="ExternalInput"),
        "uk_o": nc.dram_tensor("uk_o", [1, geom.SUK], f32,
                               kind="ExternalOutput")[:],
        "u0_o": nc.dram_tensor("u0_o", [128, geom.NBLK4], f32,
                               kind="ExternalOutput")[:],
    }
    emit(nc, t, geom)
    split_multi_waits(nc)
    return nc


def split_multi_waits(nc):
    """This env's walrus encodes at most one sem wait per instruction; hoist
    extras onto standalone EventSemaphore insts immediately before."""
    import concourse.mybir as mybir
    n = 0
    for fn in nc.m.functions:
        for bb in fn.blocks:
            insts = list(bb.instructions)
            if not any(i.sync_info and len(i.sync_info.on_wait) > 1 for i in insts):
                continue
            out = []
            for inst in insts:
                si = inst.sync_info
                if si is not None and len(si.on_wait) > 1:
                    waits = list(si.on_wait)
                    for w in waits[:-1]:
                        n += 1
                        out.append(mybir.InstEventSemaphore(
                            name=f"splitw_{n}_{inst.name}",
                            engine=inst.engine, ins=[], outs=[],
                            sync_info=mybir.SyncInfo(on_wait=[w], on_update=[]),
                        ))
                    inst.sync_info = mybir.SyncInfo(
                        on_wait=[waits[-1]], on_update=list(si.on_update))
                out.append(inst)
            bb.instructions = out
    return n


def prep_inputs(state_HS, hats, seg, W_fs, b_fs, W_fp, b_fp):
    """Shard + pack. Returns (in_maps, geom, maps)."""
    state_HS = np.asarray(state_HS, dtype=np.float32)
    hats = np.asarray(hats, dtype=np.float32)
    seg = np.asarray(seg, dtype=np.int32)
    W_fs = np.asarray(W_fs, dtype=np.float32)
    b_fs = np.asarray(b_fs, dtype=np.float32)
    W_fp = np.asarray(W_fp, dtype=np.float32)
    b_fp = np.asarray(b_fp, dtype=np.float32)

    deg = np.bincount(seg, minlength=N_NODES).astype(np.int64)
    estart = np.concatenate([[0], np.cumsum(deg)[:-1]])

    orders = []
    deg_sorted = np.empty((NC, NPER), np.int64)
    for c in range(NC):
        nodes = np.arange(c * NPER, (c + 1) * NPER)
        o = np.lexsort((nodes, -deg[nodes]))
        orders.append(nodes[o])
        deg_sorted[c] = deg[nodes[o]]
    D = deg_sorted.max(axis=0)
    geom = Geom(D)

    # constants
    w_hi = W_fs.astype(np.float16)
    w_lo = (W_fs.astype(np.float64) - w_hi.astype(np.float64)).astype(np.float16)
    ws = (w_hi, w_lo) if PHASEA_HILO else (w_hi,)
    wcat = np.empty((128, 4 * len(ws), 128), dtype=np.float16)
    for hl, w in enumerate(ws):
        for c4 in range(4):
            wcat[:, hl * 4 + c4, :] = w[c4 * 128:(c4 + 1) * 128, :]
    wcat = wcat.reshape(128, -1)
    bfs_c = _f16(b_fs.reshape(128, 1))
    wfp_c = _f16(W_fp.reshape(128, 1))
    bfp_c = np.full((128, 1), float(b_fp[0]), dtype=np.float32)

    state16 = state_HS.astype(np.float16)
    hats16 = hats.astype(np.float16)

    in_maps, maps = [], []
    for c in range(NC):
        order = orders[c]
        ordp = np.concatenate(
            [order, np.full(geom.NCHUNK - NPER, order[-1], np.int64)])
        degp = np.concatenate(
            [deg_sorted[c], np.zeros(geom.NCHUNK - NPER, np.int64)])

        # state rounds [NR, 128, 2048]
        st_p = np.empty((geom.NR, 128, 2048), np.float16)
        for k in range(geom.NR):
            rblocks = geom.block_perm[4 * k:4 * k + 4]
            nodes512 = np.concatenate(
                [ordp[128 * b:128 * (b + 1)] for b in rblocks])
            arr = state16[nodes512]                      # [512, 512]
            st_p[k] = (arr.reshape(512, 4, 128)
                       .transpose(2, 1, 0).reshape(128, 2048))

        # hats segments [NSEG, 128, SEGCOLS]
        hp = np.zeros((geom.NSEG, 128, SEGCOLS), np.float16)
        for b in range(geom.NBLK4):
            blk = geom.blocks[b]
            if blk is None:
                continue
            d, C, ng, W, gplace, ukoff = blk
            ci = 128 * b + np.arange(128)                # chunk index [ng*C]
            nodesb = ordp[ci].reshape(ng, C)
            degb = degp[ci].reshape(ng, C)
            j = np.arange(d)
            e = estart[nodesb][:, :, None] + j[None, None, :]
            valid = j[None, None, :] < degb[:, :, None]
            eidx = np.where(valid, e, 0)
            vals = hats16[eidx]                          # [ng, C, d, 128]
            for g in range(ng):
                s, off = gplace[g]
                hp[s][:, off:off + W] = vals[g].reshape(W, 128).T
        in_maps.append({
            "wcat": wcat, "bfs": bfs_c, "wfp": wfp_c, "bfp": bfp_c,
            "state_p": st_p,
            "hats_p": np.ascontiguousarray(hp),
        })
        maps.append((ordp, degp, estart[ordp]))
    return in_maps, geom, maps


def assemble(results, geom, maps):
    out = np.empty(N_NODES + N_EDGES, dtype=np.float32)
    for c in range(NC):
        ordp, degp, e0p = maps[c]
        uk = np.asarray(results[c]["uk_o"]).reshape(-1)
        u0 = np.asarray(results[c]["u0_o"])              # [128, NBLK4]
        i = np.arange(NPER)
        out[ordp[:NPER]] = u0[i % 128, i // 128]
        # uk: per block, diag output order is [chunk-in-group, group, slot]
        srcs, dsts = [], []
        for b in range(geom.NBLK4):
            blk = geom.blocks[b]
            if blk is None:
                continue
            d, C, ng, W, gplace, ukoff = blk
            ci = 128 * b + np.arange(128)
            degb = degp[ci]
            cc = np.arange(128)                          # chunk-in-block
            g = cc // C
            r = cc % C
            j = np.arange(d)
            pos = ukoff + r[:, None] * (ng * d) + g[:, None] * d + j[None, :]
            valid = j[None, :] < degb[:, None]
            if not valid.any():
                continue
            # edge index needs global estart; recompute cheaply
            srcs.append(pos[valid])
            e0 = e0p[ci]
            dsts.append((e0[:, None] + j[None, :])[valid])
        if srcs:
            sp = np.concatenate(srcs)
            dp = np.concatenate(dsts)
            out[N_NODES + dp] = 1.0 / (1.0 + np.exp(-uk[sp]))
    return out


def kernel(state_HS, hats, seg, W_fs, b_fs, W_fp, b_fp):
    from concourse.bass_utils import run_bass_kernel_spmd
    in_maps, geom, maps = prep_inputs(
        state_HS, hats, seg, W_fs, b_fs, W_fp, b_fp)
    nc = build_nc(geom)
    res = run_bass_kernel_spmd(nc, in_maps, core_ids=list(range(NC)))
    return assemble(res.results, geom, maps)


# revision 29
# speedup vs baseline: 1.0817x; 1.0196x over previous
"""Trainium2 Bass kernel for nn_PolicyNet_78365973283198 (GNN message passing).

Computation (reference):
    tempHS = tanh(state_HS @ W_fs + b_fs)          # [N, 128]
    u0     = tempHS @ W_fp + b_fp                  # [N]
    uk[e]  = <tempHS[seg[e]], hats[e]>             # [E]  (seg sorted)
    out    = sigmoid(concat([u0, uk]))             # [N + E]

Design: data-parallel over nodes on 8 cores (6250 nodes each). Nodes are
sorted by degree (descending); the degree sequence is canonicalized to the
per-rank max across cores so ONE program serves all 8 cores (~4% pad).
Blocks of 128 chunks share a uniform degree d_b (block max).

uk via ALL-PAIRS matmuls: per block, phase A produces thT [d=128, n=128]
(fp16, W hi/lo split for precision). For each group of C chunks
(C = 32/16/8 so W = C*d <= 512), ONE matmul out[c, s] = <th[g*C+c], hats[s]>
against the RAW hats tile (packed [E_DIM, slots] fp16 on host) computes every
needed dot product at 1 PE cycle/slot -- no expansion matmul, no DVE multiply,
no reduce matmul (the old pipeline cost ~2x PE + a DVE pass). The needed
values form a per-group diagonal band; Act/DVE copy the PSUM group tiles into
an SBUF stage and a single 3-dim "diagonal" DMA per block
([(L+d, C), (W, ng), (1, d)] -- partition-crossing stride on dim 0 only,
offset < row length, C <= 42: all hardware-validated) extracts the band
straight to DRAM. u0 rides on the same loaded weights as an N=1 matmul.
Host applies sigmoid to uk during unpack; u0 is sigmoided on device.
"""

import numpy as np

PHASEA_HILO = False     # hi/lo W split in phase A (2x matmuls, +precision)
N_NODES = 50000
N_EDGES = 600000
IN_DIM = 512
E_DIM = 128
NC = 8
NPER = N_NODES // NC
SEGCOLS = 4096          # hats segment width (cols) = 1MB fp16 per DMA
HB_BUFS = 6             # hats segment ring depth
HB_PREF = 4             # segments to prefetch ahead of consumption
STGW = 4352             # uk stage width (f32 cols); caps block degree at 34


def _f16(x):
    return np.ascontiguousarray(x, dtype=np.float16)


def _group_c(d):
    if d <= 16:
        return 32
    if d <= 32:
        return 16
    if d <= 64:
        return 8
    raise AssertionError(f"degree {d} > 64 unsupported")


def _zigzag(n):
    lo, hi = 0, n - 1
    out = []
    while lo <= hi:
        out.append(lo)
        if hi != lo:
            out.append(hi)
        lo += 1
        hi -= 1
    return out


class Geom:
    """Canonical (core-independent) program geometry."""

    def __init__(self, D):
        # D: canonical per-chunk degrees, len NPER, sorted descending
        NBLK = -(-NPER // 128)
        self.NR = -(-NBLK // 4)
        self.NBLK4 = 4 * self.NR
        self.NCHUNK = 128 * self.NBLK4
        Dp = np.zeros(self.NCHUNK, np.int64)
        Dp[:NPER] = D
        self.d_b = [int(Dp[128 * b]) for b in range(self.NBLK4)]
        assert 128 * max(self.d_b) <= STGW, f"max block degree {max(self.d_b)}"

        # zig-zag BLOCK order: alternate compute-dense (big d) and
        # overhead-dense (small d) blocks so DMA demand and per-block
        # engine overhead stay uniform across the run
        self.block_perm = _zigzag(self.NBLK4)
        proc = self.block_perm
        self.blocks = [None] * self.NBLK4
        seg_i, cur, ukoff = 0, 0, 0
        for b in proc:
            d = self.d_b[b]
            if d == 0:
                continue
            C = _group_c(d)
            ng = 128 // C
            W = C * d
            gplace = []
            for g in range(ng):
                if cur + W > SEGCOLS:
                    seg_i += 1
                    cur = 0
                gplace.append((seg_i, cur))
                cur += W
            self.blocks[b] = (d, C, ng, W, gplace, ukoff)
            ukoff += 128 * d
        self.NSEG = seg_i + 1
        self.SUK = max(ukoff, 1)


def emit(nc, t, geom):
    import concourse.tile as tile
    from concourse import mybir
    from concourse.ap import AP

    fp16 = mybir.dt.float16
    f32 = mybir.dt.float32
    Act = mybir.ActivationFunctionType

    wcat_d, bfs_d, wfp_d, bfp_d = t["wcat"], t["bfs"], t["wfp"], t["bfp"]
    state_d, hats_d = t["state_p"], t["hats_p"]
    uk_d, u0_d = t["uk_o"], t["u0_o"]
    NR, NSEG, NBLK4 = geom.NR, geom.NSEG, geom.NBLK4

    with tile.TileContext(nc) as tc:
        with (
            tc.tile_pool(name="const", bufs=1) as cpool,
            tc.tile_pool(name="perst", bufs=1) as ppool,
            tc.tile_pool(name="st", bufs=4) as stpool,
            tc.tile_pool(name="th", bufs=4) as thpool,
            tc.tile_pool(name="hat", bufs=HB_BUFS) as hpool,
            tc.tile_pool(name="stg", bufs=6) as sgpool,
            tc.tile_pool(name="psA", bufs=2, space="PSUM") as psA,
            tc.tile_pool(name="psK", bufs=5, space="PSUM") as psK,
            tc.tile_pool(name="psU", bufs=1, space="PSUM") as psU,
        ):
            wcat = cpool.tile([128, (8 if PHASEA_HILO else 4) * 128], fp16, tag="wcat")
            nc.sync.dma_start(wcat[:], wcat_d[:])
            bfs = cpool.tile([128, 1], fp16, tag="bfs")
            nc.sync.dma_start(bfs[:], bfs_d[:])
            wfp = cpool.tile([128, 1], fp16, tag="wfp")
            nc.sync.dma_start(wfp[:], wfp_d[:])
            bfp = cpool.tile([128, 1], f32, tag="bfp")
            nc.sync.dma_start(bfp[:], bfp_d[:])

            u0acc = psU.tile([128, NBLK4], f32, tag="u0acc")

            # hats segment ring with explicit prefetch emission
            hseg = {}
            next_seg = [0]

            def fetch_seg():
                s = next_seg[0]
                if s >= NSEG:
                    return
                tile_ = hpool.tile([128, SEGCOLS], fp16, tag="hseg")
                # SWDGE: keeps bulk loads off the sync/scalar queues so the
                # diag DMAs and Act compute never stall behind a buffer wait
                nc.gpsimd.dma_start(tile_[:], hats_d[s])
                hseg[s] = tile_
                next_seg[0] += 1

            st_tiles = {}
            next_st = [0]

            def fetch_st():
                k = next_st[0]
                if k >= NR:
                    return
                tile_ = stpool.tile([128, 2048], fp16, tag="st")
                # same SWDGE queue as hats: FIFO order == consumption order,
                # so the urgent state round is never starved by hats bulk
                nc.gpsimd.dma_start(tile_[:], state_d[k])
                st_tiles[k] = tile_
                next_st[0] += 1

            # state first: the first phase-A round must not queue behind the
            # hats prefetch burst (SDMA fair-shares rings at packet level)
            fetch_st()
            fetch_st()
            for _ in range(min(HB_PREF + 1, NSEG)):
                fetch_seg()

            cp_i = [0]          # copy engine round robin

            def copy(out_ap, in_ap):
                # weight DVE slightly more than Act (Act also does tanh+DMA)
                k = cp_i[0] % 5
                cp_i[0] += 1
                if k in (0, 1, 3):
                    nc.vector.tensor_scalar_add(out=out_ap, in0=in_ap, scalar1=0.0)
                else:
                    nc.scalar.activation(out_ap, in_ap, Act.Copy)

            npass = 8 if PHASEA_HILO else 4
            th4s = {}

            def emit_phase_a(k):
                stb = st_tiles.pop(k)
                fetch_st()
                tp = psA.tile([128, 512], f32, tag="tp")
                for i in range(npass):      # (hi/lo, c4) combos
                    c4 = i % 4
                    nc.tensor.matmul(
                        tp[:],
                        lhsT=wcat[:, i * 128:(i + 1) * 128],
                        rhs=stb[:, c4 * 512:(c4 + 1) * 512],
                        start=(i == 0), stop=(i == npass - 1),
                    )
                thT4 = thpool.tile([128, 512], fp16, tag="thT4")
                nc.scalar.activation(thT4[:], tp[:], Act.Tanh,
                                     bias=bfs[:, 0:1])
                th4s[k] = thT4

            # software pipeline: phase A of round k+1 is emitted BEFORE round
            # k's group matmuls, so when groups stall on a hats segment the
            # Tensor queue still has (already-issued) phase-A work done and
            # round k+1 starts the instant its hats arrive
            emit_phase_a(0)
            emit_phase_a(1)
            for k in range(NR):
                rblocks = geom.block_perm[4 * k:4 * k + 4]
                if k + 2 < NR:
                    emit_phase_a(k + 2)
                thT4 = th4s.pop(k)

                for b4, b in enumerate(rblocks):
                    thT = thT4[:, b4 * 128:(b4 + 1) * 128]
                    nc.tensor.matmul(u0acc[:, b:b + 1], lhsT=thT, rhs=wfp[:],
                                     start=True, stop=True)
                    blk = geom.blocks[b]
                    if blk is None:
                        continue
                    d, C, ng, W, gplace, ukoff = blk
                    LB = ng * W     # = 128*d
                    stage = sgpool.tile([128, STGW], f32, tag="stage")
                    for g in range(ng):
                        s, off = gplace[g]
                        while next_seg[0] <= s + HB_PREF:
                            if next_seg[0] >= NSEG:
                                break
                            fetch_seg()
                        hs = hseg[s]
                        pk = psK.tile([128, 512], f32, tag="pk")
                        nc.tensor.matmul(
                            pk[0:C, 0:W],
                            lhsT=thT[:, g * C:(g + 1) * C],
                            rhs=hs[:, off:off + W],
                            start=True, stop=True)
                        copy(stage[0:C, g * W:(g + 1) * W], pk[0:C, 0:W])
                    sap = stage[:]
                    diag = AP(sap.tensor, sap.offset,
                              [(STGW + d, C), (W, ng), (1, d)])
                    nc.sync.dma_start(uk_d[0:1, ukoff:ukoff + 128 * d], diag)

            u0sb = ppool.tile([128, NBLK4], f32, tag="u0sb")
            nc.scalar.activation(u0sb[:], u0acc[:], Act.Sigmoid, bias=bfp[:, 0:1])
            nc.sync.dma_start(u0_d[:], u0sb[:])
    return []


def build_nc(geom):
    import concourse.bass as bass
    from concourse import mybir

    fp16 = mybir.dt.float16
    f32 = mybir.dt.float32

    nc = bass.Bass("TRN2", target_bir_lowering=False, debug=False)
    t = {
        "wcat": nc.dram_tensor("wcat", [128, (8 if PHASEA_HILO else 4) * 128], fp16,
                               kind="ExternalInput")[:],
        "bfs": nc.dram_tensor("bfs", [128, 1], fp16, kind="ExternalInput")[:],
        "wfp": nc.dram_tensor("wfp", [128, 1], fp16, kind="ExternalInput")[:],
        "bfp": nc.dram_tensor("bfp", [128, 1], f32, kind="ExternalInput")[:],
        "state_p": nc.dram_tensor("state_p", [geom.NR, 128, 2048], fp16,
                                  kind="ExternalInput"),
        "hats_p": nc.dram_tensor("hats_p", [geom.NSEG, 128, SEGCOLS], fp16,
                                 kind="ExternalInput"),
        "uk_o": nc.dram_tensor("uk_o", [1, geom.SUK], f32,
                               kind="ExternalOutput")[:],
        "u0_o": nc.dram_tensor("u0_o", [128, geom.NBLK4], f32,
                               kind="ExternalOutput")[:],
    }
    emit(nc, t, geom)
    split_multi_waits(nc)
    return nc


def split_multi_waits(nc):
    """This env's walrus encodes at most one sem wait per instruction; hoist
    extras onto standalone EventSemaphore insts immediately before."""
    import concourse.mybir as mybir
    n = 0
    for fn in nc.m.functions:
        for bb in fn.blocks:
            insts = list(bb.instructions)
            if not any(i.sync_info and len(i.sync_info.on_wait) > 1 for i in insts):
                continue
            out = []
            for inst in insts:
                si = inst.sync_info
                if si is not None and len(si.on_wait) > 1:
                    waits = list(si.on_wait)
                    for w in waits[:-1]:
                        n += 1
                        out.append(mybir.InstEventSemaphore(
                            name=f"splitw_{n}_{inst.name}",
                            engine=inst.engine, ins=[], outs=[],
                            sync_info=mybir.SyncInfo(on_wait=[w], on_update=[]),
                        ))
                    inst.sync_info = mybir.SyncInfo(
                        on_wait=[waits[-1]], on_update=list(si.on_update))
                out.append(inst)
            bb.instructions = out
    return n


def prep_inputs(state_HS, hats, seg, W_fs, b_fs, W_fp, b_fp):
    """Shard + pack. Returns (in_maps, geom, maps)."""
    state_HS = np.asarray(state_HS, dtype=np.float32)
    hats = np.asarray(hats, dtype=np.float32)
    seg = np.asarray(seg, dtype=np.int32)
    W_fs = np.asarray(W_fs, dtype=np.float32)
    b_fs = np.asarray(b_fs, dtype=np.float32)
    W_fp = np.asarray(W_fp, dtype=np.float32)
    b_fp = np.asarray(b_fp, dtype=np.float32)

    deg = np.bincount(seg, minlength=N_NODES).astype(np.int64)
    estart = np.concatenate([[0], np.cumsum(deg)[:-1]])

    orders = []
    deg_sorted = np.empty((NC, NPER), np.int64)
    for c in range(NC):
        nodes = np.arange(c * NPER, (c + 1) * NPER)
        o = np.lexsort((nodes, -deg[nodes]))
        orders.append(nodes[o])
        deg_sorted[c] = deg[nodes[o]]
    D = deg_sorted.max(axis=0)
    geom = Geom(D)

    # constants
    w_hi = W_fs.astype(np.float16)
    w_lo = (W_fs.astype(np.float64) - w_hi.astype(np.float64)).astype(np.float16)
    ws = (w_hi, w_lo) if PHASEA_HILO else (w_hi,)
    wcat = np.empty((128, 4 * len(ws), 128), dtype=np.float16)
    for hl, w in enumerate(ws):
        for c4 in range(4):
            wcat[:, hl * 4 + c4, :] = w[c4 * 128:(c4 + 1) * 128, :]
    wcat = wcat.reshape(128, -1)
    bfs_c = _f16(b_fs.reshape(128, 1))
    wfp_c = _f16(W_fp.reshape(128, 1))
    bfp_c = np.full((128, 1), float(b_fp[0]), dtype=np.float32)

    state16 = state_HS.astype(np.float16)
    hats16 = hats.astype(np.float16)

    in_maps, maps = [], []
    for c in range(NC):
        order = orders[c]
        ordp = np.concatenate(
            [order, np.full(geom.NCHUNK - NPER, order[-1], np.int64)])
        degp = np.concatenate(
            [deg_sorted[c], np.zeros(geom.NCHUNK - NPER, np.int64)])

        # state rounds [NR, 128, 2048]
        st_p = np.empty((geom.NR, 128, 2048), np.float16)
        for k in range(geom.NR):
            rblocks = geom.block_perm[4 * k:4 * k + 4]
            nodes512 = np.concatenate(
                [ordp[128 * b:128 * (b + 1)] for b in rblocks])
            arr = state16[nodes512]                      # [512, 512]
            st_p[k] = (arr.reshape(512, 4, 128)
                       .transpose(2, 1, 0).reshape(128, 2048))

        # hats segments [NSEG, 128, SEGCOLS]
        hp = np.zeros((geom.NSEG, 128, SEGCOLS), np.float16)
        for b in range(geom.NBLK4):
            blk = geom.blocks[b]
            if blk is None:
                continue
            d, C, ng, W, gplace, ukoff = blk
            ci = 128 * b + np.arange(128)                # chunk index [ng*C]
            nodesb = ordp[ci].reshape(ng, C)
            degb = degp[ci].reshape(ng, C)
            j = np.arange(d)
            e = estart[nodesb][:, :, None] + j[None, None, :]
            valid = j[None, None, :] < degb[:, :, None]
            eidx = np.where(valid, e, 0)
            vals = hats16[eidx]                          # [ng, C, d, 128]
            for g in range(ng):
                s, off = gplace[g]
                hp[s][:, off:off + W] = vals[g].reshape(W, 128).T
        in_maps.append({
            "wcat": wcat, "bfs": bfs_c, "wfp": wfp_c, "bfp": bfp_c,
            "state_p": st_p,
            "hats_p": np.ascontiguousarray(hp),
        })
        maps.append((ordp, degp, estart[ordp]))
    return in_maps, geom, maps


def assemble(results, geom, maps):
    out = np.empty(N_NODES + N_EDGES, dtype=np.float32)
    for c in range(NC):
        ordp, degp, e0p = maps[c]
        uk = np.asarray(results[c]["uk_o"]).reshape(-1)
        u0 = np.asarray(results[c]["u0_o"])              # [128, NBLK4]
        i = np.arange(NPER)
        out[ordp[:NPER]] = u0[i % 128, i // 128]
        # uk: per block, diag output order is [chunk-in-group, group, slot]
        srcs, dsts = [], []
        for b in range(geom.NBLK4):
            blk = geom.blocks[b]
            if blk is None:
                continue
            d, C, ng, W, gplace, ukoff = blk
            ci = 128 * b + np.arange(128)
            degb = degp[ci]
            cc = np.arange(128)                          # chunk-in-block
            g = cc // C
            r = cc % C
            j = np.arange(d)
            pos = ukoff + r[:, None] * (ng * d) + g[:, None] * d + j[None, :]
            valid = j[None, :] < degb[:, None]
            if not valid.any():
                continue
            # edge index needs global estart; recompute cheaply
            srcs.append(pos[valid])
            e0 = e0p[ci]
            dsts.append((e0[:, None] + j[None, :])[valid])
        if srcs:
            sp = np.concatenate(srcs)
            dp = np.concatenate(dsts)
            out[N_NODES + dp] = 1.0 / (1.0 + np.exp(-uk[sp]))
    return out


def kernel(state_HS, hats, seg, W_fs, b_fs, W_fp, b_fp):
    from concourse.bass_utils import run_bass_kernel_spmd
    in_maps, geom, maps = prep_inputs(
        state_HS, hats, seg, W_fs, b_fs, W_fp, b_fp)
    nc = build_nc(geom)
    res = run_bass_kernel_spmd(nc, in_maps, core_ids=list(range(NC)))
    return assemble(res.results, geom, maps)
